# revision 3
# baseline (speedup 1.0000x reference)
"""ANI-style MoE routing kernel for 8 Trainium2 NeuronCores.

Strategy (data-parallel + host routing):
  - Host: sort atoms by type, split each type's atom list evenly across the
    8 cores, and build per-core per-expert contiguous batches padded to a
    fixed capacity CAP.  Batches are stored pre-transposed (feature-major,
    [384, 4*CAP]) so the device streams them straight into matmuls.
  - Device (per core): for each expert, a 3-layer MLP in feature-major
    layout using float32r matmuls (full-rate fp32 at N>=256).  CELU is
    computed exactly as  celu(z) = max(z, min(exp(z)-1, 0))  split across
    ScalarE (exp), GpSimd (min/add fuse) and VectorE (max).  Layer 3
    (H2 -> 1) accumulates all per-atom energies of all chunks into a single
    [1, 512] PSUM tile; one final free-axis reduce yields the core's
    partial energy.
  - Host: sum the 8 partial scalars (+ exact f64 corrections for padding
    rows and the b3 bias, both computable host-side from the weights).

Zero-padding rows pass through the MLP to exactly zero energy when
b1 == b2 == 0 (always true for this problem's init); the general-bias path
adds per-layer bias matmuls and corrects the pad contribution on the host.
"""

import os
import sys

import numpy as np

try:
    import concourse.bass as bass  # noqa: F401
except ImportError:  # pragma: no cover
    sys.path.insert(0, "/opt/trn_rl_repo")
    import concourse.bass as bass  # noqa: F401

import concourse.mybir as mybir
import concourse.tile as tile
from concourse import bacc
from concourse import bass_utils

IN_DIM = 384
H1 = 192
H2 = 160
E = 4
N_CORES = 8
N_ATOMS = 262144

CHUNK = 512
CAP = 8704  # per-(core, expert) atom capacity; 17 chunks of 512
N_CHUNKS = CAP // CHUNK

F32 = mybir.dt.float32
F32R = mybir.dt.float32r
AF = mybir.ActivationFunctionType
ALU = mybir.AluOpType


def _build_graph(with_bias: bool):
    nc = bacc.Bacc(
        "TRN2",
        target_bir_lowering=False,
        debug=False,
        enable_asserts=False,
        num_devices=N_CORES,
    )
    xT = nc.dram_tensor("xT", [IN_DIM, E * CAP], F32, kind="ExternalInput").ap()
    W1 = nc.dram_tensor("W1", [E, IN_DIM, H1], F32, kind="ExternalInput").ap()
    W2 = nc.dram_tensor("W2", [E, H1, H2], F32, kind="ExternalInput").ap()
    W3 = nc.dram_tensor("W3", [E, H2, 1], F32, kind="ExternalInput").ap()
    B1 = B2 = None
    if with_bias:
        B1 = nc.dram_tensor("B1", [E, H1], F32, kind="ExternalInput").ap()
        B2 = nc.dram_tensor("B2", [E, H2], F32, kind="ExternalInput").ap()
    out = nc.dram_tensor("out", [1, 1], F32, kind="ExternalOutput").ap()

    with tile.TileContext(nc) as tc:
        _emit(tc, xT, W1, W2, W3, B1, B2, out, with_bias)
    nc.compile()
    return nc


def _emit(tc, xT, W1, W2, W3, B1, B2, out, with_bias):
    nc = tc.nc
    xT3 = xT.rearrange("(kt kp) n -> kp kt n", kp=128)  # [128, 3, E*CAP]

    with (
        tc.tile_pool(name="wpool", bufs=2) as wp,
        tc.tile_pool(name="xpool", bufs=3) as xp,
        tc.tile_pool(name="hpool", bufs=3) as hp,
        tc.tile_pool(name="gpool", bufs=2) as gp,
        tc.tile_pool(name="zpool", bufs=2, space="PSUM") as zp,
        tc.tile_pool(name="z2pool", bufs=1, space="PSUM") as z2p,
        tc.tile_pool(name="accpool", bufs=1, space="PSUM") as accp,
        tc.tile_pool(name="cpool", bufs=1) as cp,
    ):
        acc = accp.tile([1, CHUNK], F32, tag="acc")  # persistent energy acc
        ones = None
        if with_bias:
            ones = cp.tile([1, CHUNK], F32R, tag="ones")
            nc.vector.memset(ones[:], 1.0)

        first_mm3 = [True]

        for e in range(E):
            # --- expert weights to SBUF ---
            w1s = wp.tile([128, 3, H1], F32R, tag="w1")  # k-tiles of W1[e]
            nc.sync.dma_start(
                out=w1s[:], in_=W1[e].rearrange("(kt kp) m -> kp kt m", kp=128).bitcast(F32R)
            )
            w2s = wp.tile([128, 2, H2], F32R, tag="w2")
            nc.sync.dma_start(out=w2s[:, 0, :], in_=W2[e][0:128, :].bitcast(F32R))
            nc.sync.dma_start(out=w2s[0:64, 1, :], in_=W2[e][128:192, :].bitcast(F32R))
            w3s = wp.tile([128, 2], F32R, tag="w3")
            nc.sync.dma_start(out=w3s[:, 0:1], in_=W3[e][0:128, :].bitcast(F32R))
            nc.sync.dma_start(out=w3s[0:32, 1:2], in_=W3[e][128:160, :].bitcast(F32R))
            if with_bias:
                b1s = wp.tile([1, H1], F32R, tag="b1")
                nc.sync.dma_start(out=b1s[:], in_=B1[e : e + 1, :].bitcast(F32R))
                b2s = wp.tile([1, H2], F32R, tag="b2")
                nc.sync.dma_start(out=b2s[:], in_=B2[e : e + 1, :].bitcast(F32R))

            for c in range(N_CHUNKS):
                off = e * CAP + c * CHUNK
                xa = xp.tile([128, 3, CHUNK], F32R, tag="xa")
                nc.sync.dma_start(out=xa[:], in_=xT3[:, :, off : off + CHUNK].bitcast(F32R))

                # ---- layer 1: z1 = x @ W1 (+b1)   [192 rows, CHUNK cols] ----
                z1 = zp.tile([128, 2 * CHUNK], F32, tag="z1")
                m_specs1 = [(0, 128, z1[:, 0:CHUNK]), (128, 64, z1[0:64, CHUNK:])]
                for m0, msz, zslice in m_specs1:
                    for kt in range(3):
                        nc.tensor.matmul(
                            zslice,
                            lhsT=w1s[:, kt, m0 : m0 + msz],
                            rhs=xa[:, kt, :],
                            start=(kt == 0),
                            stop=(kt == 2 and not with_bias),
                        )
                    if with_bias:
                        nc.tensor.matmul(
                            zslice,
                            lhsT=b1s[:, m0 : m0 + msz],
                            rhs=ones[:],
                            start=False,
                            stop=True,
                        )

                # ---- celu: g = max(z, min(exp(z)-1, 0)) ----
                g1a = gp.tile([128, CHUNK], F32R, tag="g1a")
                g1b = gp.tile([64, CHUNK], F32R, tag="g1b")
                for (m0, msz, zslice), g in zip(m_specs1, (g1a, g1b)):
                    t = hp.tile([128, CHUNK], F32, tag="t")
                    nc.scalar.activation(t[0:msz, :], zslice, AF.Exp)
                    u = hp.tile([128, CHUNK], F32, tag="u")
                    nc.gpsimd.tensor_scalar(
                        out=u[0:msz, :],
                        in0=t[0:msz, :],
                        scalar1=-1.0,
                        scalar2=0.0,
                        op0=ALU.add,
                        op1=ALU.min,
                    )
                    nc.vector.tensor_tensor(
                        out=g[0:msz, :], in0=zslice, in1=u[0:msz, :], op=ALU.max
                    )

                # ---- layer 2: z2 = g1 @ W2 (+b2)   [160 rows, CHUNK cols] ----
                z2 = z2p.tile([128, 2 * CHUNK], F32, tag="z2")
                m_specs2 = [(0, 128, z2[:, 0:CHUNK]), (128, 32, z2[0:32, CHUNK:])]
                for m0, msz, zslice in m_specs2:
                    nc.tensor.matmul(
                        zslice,
                        lhsT=w2s[:, 0, m0 : m0 + msz],
                        rhs=g1a[:],
                        start=True,
                        stop=False,
                    )
                    nc.tensor.matmul(
                        zslice,
                        lhsT=w2s[0:64, 1, m0 : m0 + msz],
                        rhs=g1b[:],
                        start=False,
                        stop=(not with_bias),
                    )
                    if with_bias:
                        nc.tensor.matmul(
                            zslice,
                            lhsT=b2s[:, m0 : m0 + msz],
                            rhs=ones[:],
                            start=False,
                            stop=True,
                        )

                g2a = gp.tile([128, CHUNK], F32R, tag="g2a")
                g2b = gp.tile([32, CHUNK], F32R, tag="g2b")
                for (m0, msz, zslice), g in zip(m_specs2, (g2a, g2b)):
                    t = hp.tile([128, CHUNK], F32, tag="t")
                    nc.scalar.activation(t[0:msz, :], zslice, AF.Exp)
                    u = hp.tile([128, CHUNK], F32, tag="u")
                    nc.gpsimd.tensor_scalar(
                        out=u[0:msz, :],
                        in0=t[0:msz, :],
                        scalar1=-1.0,
                        scalar2=0.0,
                        op0=ALU.add,
                        op1=ALU.min,
                    )
                    nc.vector.tensor_tensor(
                        out=g[0:msz, :], in0=zslice, in1=u[0:msz, :], op=ALU.max
                    )

                # ---- layer 3: acc[1, CHUNK] += g2 @ W3[e] ----
                last = e == E - 1 and c == N_CHUNKS - 1
                nc.tensor.matmul(
                    acc[:],
                    lhsT=w3s[:, 0:1],
                    rhs=g2a[:],
                    start=first_mm3[0],
                    stop=False,
                )
                first_mm3[0] = False
                nc.tensor.matmul(
                    acc[:],
                    lhsT=w3s[0:32, 1:2],
                    rhs=g2b[:],
                    start=False,
                    stop=last,
                )

        res = cp.tile([1, 1], F32, tag="res")
        nc.vector.tensor_reduce(
            out=res[:], in_=acc[:], axis=mybir.AxisListType.X, op=ALU.add
        )
        nc.sync.dma_start(out=out, in_=res[:])


_GRAPH_CACHE = {}


def _get_graph(with_bias: bool):
    if with_bias not in _GRAPH_CACHE:
        _GRAPH_CACHE[with_bias] = _build_graph(with_bias)
    return _GRAPH_CACHE[with_bias]


def _celu64(v):
    return np.where(v > 0, v, np.expm1(np.minimum(v, 0.0)))


def kernel(aev_inputs, atom_types, W1, b1, W2, b2, W3, b3):
    aev = np.asarray(aev_inputs, dtype=np.float32)
    types = np.asarray(atom_types).astype(np.int64)
    W1 = np.ascontiguousarray(np.asarray(W1, dtype=np.float32))
    b1 = np.asarray(b1, dtype=np.float32)
    W2 = np.ascontiguousarray(np.asarray(W2, dtype=np.float32))
    b2 = np.asarray(b2, dtype=np.float32)
    W3 = np.ascontiguousarray(np.asarray(W3, dtype=np.float32))
    b3 = np.asarray(b3, dtype=np.float32)

    with_bias = bool(np.any(b1) or np.any(b2))

    # ---- host routing: per-type atom lists, split evenly over cores ----
    order = np.argsort(types, kind="stable")
    sorted_types = types[order]
    bounds = np.searchsorted(sorted_types, np.arange(E + 1))
    type_lists = [order[bounds[e] : bounds[e + 1]] for e in range(E)]

    n_real = np.zeros((N_CORES, E), dtype=np.int64)
    in_maps = []
    for c in range(N_CORES):
        xcT = np.zeros((IN_DIM, E * CAP), dtype=np.float32)
        for e in range(E):
            lst = type_lists[e]
            lo = (len(lst) * c) // N_CORES
            hi = (len(lst) * (c + 1)) // N_CORES
            idx = lst[lo:hi]
            n = len(idx)
            assert n <= CAP, f"capacity overflow: core {c} expert {e} has {n} > {CAP}"
            n_real[c, e] = n
            xcT[:, e * CAP : e * CAP + n] = aev[idx].T
        m = {"xT": xcT, "W1": W1, "W2": W2, "W3": W3}
        if with_bias:
            m["B1"] = np.ascontiguousarray(b1)
            m["B2"] = np.ascontiguousarray(b2)
        in_maps.append(m)

    nc = _get_graph(with_bias)
    results = bass_utils.run_bass_kernel_spmd(
        nc, in_maps, core_ids=list(range(N_CORES))
    ).results

    total = 0.0
    for c in range(N_CORES):
        total += float(results[c]["out"][0, 0])

    # ---- exact host-side corrections (float64) ----
    # device computes sum over ALL rows (incl. pads) of  g2 @ W3  (no b3).
    # pad rows are zero vectors -> contribute y0dev_e each.
    counts_e = n_real.sum(axis=0)  # real atoms per expert
    pads_e = N_CORES * CAP - counts_e
    for e in range(E):
        h1 = _celu64(b1[e].astype(np.float64))
        z2 = h1 @ W2[e].astype(np.float64) + b2[e].astype(np.float64)
        y0dev = _celu64(z2) @ W3[e].astype(np.float64)[:, 0]
        total -= float(pads_e[e]) * float(y0dev)
        total += float(counts_e[e]) * float(b3[e][0])

    return np.asarray(total, dtype=np.float32)


# revision 26
# speedup vs baseline: 42518.3319x; 42518.3319x over previous
"""ANI-style MoE routing kernel for 8 Trainium2 NeuronCores.

Strategy (data-parallel + host routing):
  - Host: sort atoms by type, split each type's atom list evenly across the
    8 cores, and build per-core per-expert contiguous batches padded to
    adaptive per-expert capacities (multiples of 512).  Batches are stored
    pre-transposed (feature-major, [384, sum(caps)]) in bf16 so the device
    streams them straight into matmuls.
  - Device (per core): for each expert, a 3-layer MLP in feature-major
    layout with bf16 matmul operands (f32 PSUM accumulate), emitted as a
    2-stage software pipeline so the PE never queue-stalls on pointwise ops.
    CELU is exact:
      layer 1:  g = max(z, min(exp(z)-1, 0))     [ACT exp -> DVE ts -> DVE tt]
      layer 2:  h = relu(z) + min(exp(z)-1, 0)   [r on DVE; q's main half is
                computed on ACT as relu(1-exp(z)) = -q with the negation
                folded into a negated-W3 lhsT in layer 3; the r+q add is
                folded into layer 3's matmul as two rhs parts]
    exp outputs stay f32 (bf16 near 1.0 would wreck expm1 precision);
    pointwise ops run on packed [128, 1024] PSUM views (the m-tile pair),
    tolerating never-read garbage lanes.  Layer 3 accumulates all per-atom
    energies into one [1, 512] PSUM tile across all chunks; a single
    free-axis reduce yields the core's partial energy.
  - Host: sum the 8 partial scalars (+ exact f64 corrections for padding
    rows and the b3 bias, both computable host-side from the weights).

Zero-padding rows pass through the MLP to exactly zero energy when
b1 == b2 == 0 (always true for this problem's init); the general-bias path
adds per-layer bias matmuls and corrects the pad contribution on the host.
"""

import os
import sys

import numpy as np

try:
    import concourse.bass as bass  # noqa: F401
except ImportError:  # pragma: no cover
    sys.path.insert(0, "/opt/trn_rl_repo")
    import concourse.bass as bass  # noqa: F401

import concourse.mybir as mybir
import concourse.tile as tile
from concourse import bacc
from concourse import bass_utils

IN_DIM = 384
H1 = 192
H2 = 160
E = 4
N_CORES = 8
N_ATOMS = 262144

CHUNK = 512

F32 = mybir.dt.float32
F32R = mybir.dt.float32r
BF16 = mybir.dt.bfloat16
AF = mybir.ActivationFunctionType
ALU = mybir.AluOpType

USE_BF16 = os.environ.get("BF16", "1") == "1"
MMDT = BF16 if USE_BF16 else F32R  # matmul operand dtype
IODT_NP = None  # set lazily in prepare_in_maps


def _np_mmdt():
    if USE_BF16:
        import ml_dtypes

        return ml_dtypes.bfloat16
    return np.float32


def _build_graph(with_bias: bool, caps, repeat: int = 1):
    nc = bacc.Bacc(
        "TRN2",
        target_bir_lowering=False,
        debug=False,
        enable_asserts=False,
        num_devices=N_CORES,
    )
    total_cap = sum(caps)
    iodt = BF16 if USE_BF16 else F32
    xT = nc.dram_tensor("xT", [IN_DIM, total_cap], iodt, kind="ExternalInput").ap()
    W1 = nc.dram_tensor("W1", [E, IN_DIM, H1], iodt, kind="ExternalInput").ap()
    W2 = nc.dram_tensor("W2", [E, H1, H2], iodt, kind="ExternalInput").ap()
    W3 = nc.dram_tensor("W3", [E, H2, 1], iodt, kind="ExternalInput").ap()
    B1 = B2 = None
    if with_bias:
        bdt = BF16 if USE_BF16 else F32
        B1 = nc.dram_tensor("B1", [E, H1], bdt, kind="ExternalInput").ap()
        B2 = nc.dram_tensor("B2", [E, H2], bdt, kind="ExternalInput").ap()
    out = nc.dram_tensor("out", [1, 1], F32, kind="ExternalOutput").ap()

    with tile.TileContext(nc) as tc:
        _emit(tc, xT, W1, W2, W3, B1, B2, out, with_bias, caps, repeat)
    nc.compile()
    return nc


def _emit(tc, xT, W1, W2, W3, B1, B2, out, with_bias, caps, repeat=1):
    nc = tc.nc
    xT3 = xT.rearrange("(kt kp) n -> kp kt n", kp=128)  # [128, 3, E*CAP]

    with (
        tc.tile_pool(name="wpool", bufs=2) as wp,
        tc.tile_pool(name="xpool", bufs=4) as xp,
        tc.tile_pool(name="hpool", bufs=6) as hp,
        tc.tile_pool(name="gpool", bufs=3) as gp,
        tc.tile_pool(
            name="zpool", bufs=int(os.environ.get("Z1_BUFS", "1")), space="PSUM"
        ) as zp,
        tc.tile_pool(
            name="z2pool", bufs=int(os.environ.get("Z2_BUFS", "2")), space="PSUM"
        ) as z2p,
        tc.tile_pool(name="accpool", bufs=1, space="PSUM") as accp,
        tc.tile_pool(name="cpool", bufs=1) as cp,
    ):
        import contextlib

        loop_cm = tc.For_i(0, repeat, 1) if repeat > 1 else contextlib.nullcontext()
        with loop_cm:
            _emit_body(
                tc, xT3, W1, W2, W3, B1, B2, out, with_bias, caps,
                wp, xp, hp, gp, zp, z2p, accp, cp,
            )


def _emit_body(
    tc, xT3, W1, W2, W3, B1, B2, out, with_bias, caps,
    wp, xp, hp, gp, zp, z2p, accp, cp,
):
    nc = tc.nc
    acc = accp.tile([1, CHUNK], F32, tag="acc")  # persistent energy accumulator
    ones = None
    if with_bias:
        ones = cp.tile([1, CHUNK], MMDT, tag="ones")
        nc.vector.memset(ones[:], 1.0)

    celu_mode = os.environ.get("CELU_MODE", "full")

    def celu_max(zfull, g):
        """g = celu(z) = max(z, min(exp(z)-1, 0)) on a packed [128, 2*CHUNK]
        view (garbage lanes tolerated; never read downstream)."""
        if celu_mode == "copy":  # timing-skeleton variant (wrong numerics)
            nc.scalar.activation(g[:], zfull, AF.Copy)
            return
        t = hp.tile([128, 2 * CHUNK], F32, tag="t")
        nc.scalar.activation(t[:], zfull, AF.Exp)
        if os.environ.get("U1ACT", "0") == "1":
            # un = relu(1 - exp(z)) = -min(exp(z)-1, 0) on ACT;
            # fused DVE scalar_tensor_tensor computes g = max(z, -un)
            un = hp.tile([128, 2 * CHUNK], F32, tag="u")
            nc.scalar.activation(un[:], t[:], AF.Relu, bias=1.0, scale=-1.0)
            nc.vector.scalar_tensor_tensor(
                out=g[:], in0=un[:], scalar=-1.0, in1=zfull,
                op0=ALU.mult, op1=ALU.max,
            )
        else:
            u = hp.tile([128, 2 * CHUNK], F32, tag="u")
            nc.vector.tensor_scalar(
                out=u[:], in0=t[:],
                scalar1=-1.0, scalar2=0.0, op0=ALU.add, op1=ALU.min,
            )
            nc.vector.tensor_tensor(out=g[:], in0=zfull, in1=u[:], op=ALU.max)

    def celu_add(zfull, r, q, idx=0):
        """celu(z) = r + q with r = relu(z), q = min(exp(z)-1, 0).
        The r+q add is folded into the consuming matmul (two rhs parts).
        r alternates ACT/DVE per chunk to balance engine load."""
        if celu_mode == "copy":
            nc.scalar.activation(r[:], zfull, AF.Copy)
            nc.vector.memset(q[:], 0.0)
            return
        t = hp.tile([128, 2 * CHUNK], F32, tag="t")
        nc.scalar.activation(t[:], zfull, AF.Exp)
        if idx % 2 == 0:
            nc.scalar.activation(r[:], zfull, AF.Relu)
        else:
            nc.vector.tensor_scalar(
                out=r[:], in0=zfull, scalar1=0.0, scalar2=None,
                op0=ALU.max, op1=ALU.bypass,
            )
        nc.vector.tensor_scalar(
            out=q[:], in0=t[:],
            scalar1=-1.0, scalar2=0.0, op0=ALU.add, op1=ALU.min,
        )

    only = os.environ.get("ONLY", "full")  # dma | mm1 | full  (bisection modes)
    ecap_off = [0]
    for e in range(E):
        ecap_off.append(ecap_off[-1] + caps[e])
    chunks = [(e, c) for e in range(E) for c in range(caps[e] // CHUNK)]
    n = len(chunks)
    S = {}  # software-pipeline state per chunk index
    weights = None  # tiles of the expert currently being loaded (stage A)
    first_mm3 = [True]

    for i in range(n + 2):
        # ---- stage A(i): weights + x DMA, mm1, celu1 ----
        if i < n:
            e, c = chunks[i]
            if c == 0:
                w1s = wp.tile([128, 3, H1], MMDT, tag="w1")
                nc.sync.dma_start(
                    out=w1s[:],
                    in_=W1[e].rearrange("(kt kp) m -> kp kt m", kp=128).bitcast(MMDT),
                )
                w2s = wp.tile([128, 2, H2], MMDT, tag="w2")
                nc.sync.dma_start(out=w2s[:, 0, :], in_=W2[e][0:128, :].bitcast(MMDT))
                nc.sync.dma_start(
                    out=w2s[0:64, 1, :], in_=W2[e][128:192, :].bitcast(MMDT)
                )
                w3s = wp.tile([128, 2], MMDT, tag="w3")
                nc.sync.dma_start(out=w3s[:, 0:1], in_=W3[e][0:128, :].bitcast(MMDT))
                nc.sync.dma_start(out=w3s[0:32, 1:2], in_=W3[e][128:160, :].bitcast(MMDT))
                if with_bias:
                    b1s = wp.tile([1, H1], MMDT, tag="b1")
                    nc.sync.dma_start(out=b1s[:], in_=B1[e : e + 1, :].bitcast(MMDT))
                    b2s = wp.tile([1, H2], MMDT, tag="b2")
                    nc.sync.dma_start(out=b2s[:], in_=B2[e : e + 1, :].bitcast(MMDT))
                else:
                    b1s = b2s = None
                weights = (w1s, w2s, w3s, b1s, b2s)

            w1s, w2s, w3s, b1s, b2s = weights
            off = ecap_off[e] + c * CHUNK
            xa = xp.tile([128, 3, CHUNK], MMDT, tag="xa")
            nc.sync.dma_start(
                out=xa[:], in_=xT3[:, :, off : off + CHUNK].bitcast(MMDT)
            )
            if only == "dma":
                continue

            z1 = zp.tile([128, 2 * CHUNK], F32, tag="z1")
            m_specs1 = [(0, 128, z1[:, 0:CHUNK]), (128, 64, z1[0:64, CHUNK:])]
            for m0, msz, zslice in m_specs1:
                for kt in range(3):
                    nc.tensor.matmul(
                        zslice,
                        lhsT=w1s[:, kt, m0 : m0 + msz],
                        rhs=xa[:, kt, :],
                        start=(kt == 0),
                        stop=(kt == 2 and not with_bias),
                    )
                if with_bias:
                    nc.tensor.matmul(
                        zslice, lhsT=b1s[:, m0 : m0 + msz], rhs=ones[:],
                        start=False, stop=True,
                    )
            if only == "mm1":
                continue
            if os.environ.get("CELU1", "max") == "add":
                r1 = gp.tile([128, 2 * CHUNK], MMDT, tag="g1")
                q1 = gp.tile([128, 2 * CHUNK], MMDT, tag="q1")
                celu_add(z1[:], r1, q1, idx=i + 1)
                S[i] = {"g1": r1, "q1": q1, "w": weights}
            else:
                g1 = gp.tile([128, 2 * CHUNK], MMDT, tag="g1")
                celu_max(z1[:], g1)
                S[i] = {"g1": g1, "w": weights}

        # ---- stage B(i-1): mm2, celu2 ----
        j = i - 1
        if 0 <= j < n and only == "full":
            st = S[j]
            w1s, w2s, w3s, b1s, b2s = st["w"]
            g1 = st["g1"]
            z2 = z2p.tile([128, 2 * CHUNK], F32, tag="z2")
            m_specs2 = [(0, 128, z2[:, 0:CHUNK]), (128, 32, z2[0:32, CHUNK:])]
            g1_parts = [g1] + ([st["q1"]] if "q1" in st else [])
            for m0, msz, zslice in m_specs2:
                first = True
                for gi, gpart in enumerate(g1_parts):
                    last_part = gi == len(g1_parts) - 1
                    nc.tensor.matmul(
                        zslice, lhsT=w2s[:, 0, m0 : m0 + msz],
                        rhs=gpart[0:128, 0:CHUNK], start=first, stop=False,
                    )
                    first = False
                    nc.tensor.matmul(
                        zslice, lhsT=w2s[0:64, 1, m0 : m0 + msz],
                        rhs=gpart[0:64, CHUNK : 2 * CHUNK],
                        start=False, stop=(last_part and not with_bias),
                    )
                if with_bias:
                    nc.tensor.matmul(
                        zslice, lhsT=b2s[:, m0 : m0 + msz], rhs=ones[:],
                        start=False, stop=True,
                    )
            r2 = gp.tile([128, 2 * CHUNK], MMDT, tag="r2")
            q2 = gp.tile([128, 2 * CHUNK], MMDT, tag="q2")
            celu_add(z2[:], r2, q2, idx=j)
            st["r2"], st["q2"] = r2, q2

        # ---- stage C(i-2): mm3 accumulate ----
        k = i - 2
        if 0 <= k < n and only == "full":
            st = S.pop(k)
            w3s = st["w"][2]
            for part in ("r2", "q2"):
                g2 = st[part]
                nc.tensor.matmul(
                    acc[:], lhsT=w3s[:, 0:1], rhs=g2[0:128, 0:CHUNK],
                    start=first_mm3[0], stop=False,
                )
                first_mm3[0] = False
                nc.tensor.matmul(
                    acc[:], lhsT=w3s[0:32, 1:2], rhs=g2[0:32, CHUNK : 2 * CHUNK],
                    start=False, stop=(k == n - 1 and part == "q2"),
                )

    res = cp.tile([1, 1], F32, tag="res")
    if only == "full":
        nc.vector.tensor_reduce(
            out=res[:], in_=acc[:], axis=mybir.AxisListType.X, op=ALU.add
        )
    else:
        nc.vector.memset(res[:], 0.0)
    nc.sync.dma_start(out=out, in_=res[:])


_GRAPH_CACHE = {}


def _get_graph(with_bias: bool, caps):
    key = (with_bias, tuple(caps))
    if key not in _GRAPH_CACHE:
        _GRAPH_CACHE[key] = _build_graph(with_bias, caps)
    return _GRAPH_CACHE[key]


def _celu64(v):
    return np.where(v > 0, v, np.expm1(np.minimum(v, 0.0)))


def prepare_in_maps(aev_inputs, atom_types, W1, b1, W2, b2, W3, b3):
    """Host routing: build per-core input maps + metadata for corrections."""
    ndt = _np_mmdt()
    aev = np.asarray(aev_inputs, dtype=np.float32)
    types = np.asarray(atom_types).astype(np.int64)
    W1f = np.asarray(W1, dtype=np.float32)
    b1 = np.asarray(b1, dtype=np.float32)
    W2f = np.asarray(W2, dtype=np.float32)
    b2 = np.asarray(b2, dtype=np.float32)
    W3f = np.asarray(W3, dtype=np.float32)
    b3 = np.asarray(b3, dtype=np.float32)
    W1 = np.ascontiguousarray(W1f.astype(ndt))
    W2 = np.ascontiguousarray(W2f.astype(ndt))
    W3 = np.ascontiguousarray(W3f.astype(ndt))

    with_bias = bool(np.any(b1) or np.any(b2))

    # per-type atom lists, split evenly over cores
    order = np.argsort(types, kind="stable")
    sorted_types = types[order]
    bounds = np.searchsorted(sorted_types, np.arange(E + 1))
    type_lists = [order[bounds[e] : bounds[e + 1]] for e in range(E)]

    # per-(core, expert) slices and adaptive per-expert capacities
    slices = [[None] * E for _ in range(N_CORES)]
    n_real = np.zeros((N_CORES, E), dtype=np.int64)
    for e in range(E):
        lst = type_lists[e]
        for c in range(N_CORES):
            lo = (len(lst) * c) // N_CORES
            hi = (len(lst) * (c + 1)) // N_CORES
            slices[c][e] = lst[lo:hi]
            n_real[c, e] = hi - lo
    caps = tuple(
        int(-(-int(n_real[:, e].max()) // CHUNK) * CHUNK) for e in range(E)
    )
    offs = [0]
    for e in range(E):
        offs.append(offs[-1] + caps[e])

    in_maps = []
    for c in range(N_CORES):
        xcT = np.zeros((IN_DIM, offs[-1]), dtype=ndt)
        for e in range(E):
            idx = slices[c][e]
            xcT[:, offs[e] : offs[e] + len(idx)] = aev[idx].T.astype(ndt)
        m = {"xT": xcT, "W1": W1, "W2": W2, "W3": W3}
        if with_bias:
            m["B1"] = np.ascontiguousarray(b1.astype(ndt))
            m["B2"] = np.ascontiguousarray(b2.astype(ndt))
        in_maps.append(m)
    return in_maps, n_real, with_bias, (b1, W2f, b2, W3f, b3), caps


def postprocess(results, n_real, wdata, caps):
    """Sum core partials + exact f64 corrections for pads and b3."""
    b1, W2, b2, W3, b3 = wdata
    total = 0.0
    for c in range(N_CORES):
        total += float(results[c]["out"][0, 0])
    counts_e = n_real.sum(axis=0)
    pads_e = np.array([N_CORES * caps[e] - counts_e[e] for e in range(E)])
    for e in range(E):
        h1 = _celu64(b1[e].astype(np.float64))
        z2 = h1 @ W2[e].astype(np.float64) + b2[e].astype(np.float64)
        y0dev = _celu64(z2) @ W3[e].astype(np.float64)[:, 0]
        total -= float(pads_e[e]) * float(y0dev)
        total += float(counts_e[e]) * float(b3[e][0])
    return np.asarray(total, dtype=np.float32)


def kernel(aev_inputs, atom_types, W1, b1, W2, b2, W3, b3):
    in_maps, n_real, with_bias, wdata, caps = prepare_in_maps(
        aev_inputs, atom_types, W1, b1, W2, b2, W3, b3
    )
    nc = _get_graph(with_bias, caps)
    results = bass_utils.run_bass_kernel_spmd(
        nc, in_maps, core_ids=list(range(N_CORES))
    ).results
    return postprocess(results, n_real, wdata, caps)
